# revision 1
# baseline (speedup 1.0000x reference)
"""Trainium2 Bass kernel for AttentionMLP (nn_AttentionMLP_72997264163220).

Reference computation:
  k/q/v = x @ W{k,q,v}.T + b      (D=3800 -> D)
  scores = q @ k.T / sqrt(D); attn = softmax(scores, -1)
  attended = attn @ v; h = attended.mean(seq)
  h = sigmoid(h @ W1.T + b1); h = sigmoid(h @ W2.T + b2); out = h @ W3.T + b3

Key algebraic simplification: the mean over the sequence commutes with
the attention matmul and the (linear) v projection,
  h = mean_i(attn) @ v = (abar @ x) @ Wv.T + bv,   abar = colmean_i(attn)
so v is never materialized: one [S]-vector per batch contracts x down to
a single [D]-vector before touching Wv. This removes ~1/3 of the matmul
work vs the naive dataflow.

Sharding: data-parallel over batch. 16 batches -> 8 cores x 2 batches
(512 tokens per core). All weights replicated, host pre-transposed /
tiled / cast. Big matmuls in bf16 (fp32 PSUM accumulate); softmax and
the MLP in fp32.

Device dataflow per core (SBUF partition dim always first; D padded to
3840 = 30*128 with a bias feature at d=3800):
  xT    [128, 30, 512] bf16  x^T (dp, kc, token); row d=3800 == 1
  x_tok [128, 4, 3840] bf16  x (token_p, token-tile, d); col d=3800 == 1
  per e-tile et in 30:  k_et/q_et [128,512] bf16  (q pre-scaled 1/sqrt(D))
     scores[2b+it] psum [128,256] += q_et_slice^T @ k_et_slice  over et
  softmax rows (fp32, on ACT/DVE) -> attn bf16 [128(i), 256(j)]
  abar[b] = colsum_i(attn)/S  via matmul with a const 1/S vector
  xa[b]   = abar[b] @ x       via x_tok   -> xaT [128, 30, 2] bf16 (xa[3800]=1)
  hT[et]  = Wv_tile^T @ xaT   (Wv has unit row at e=3800 -> hT[3800]=1)
  fp32 MLP; biases via the unit feature / unit rows, so no unaligned
  single-partition writes are ever needed.
"""

import sys
import types

import numpy as np

if "/opt/trn_rl_repo" not in sys.path:
    sys.path.insert(0, "/opt/trn_rl_repo")


# ---------------------------------------------------------------------------
# NTFF profile hook shim (antenv.axon_hooks is absent in this image). Needed
# only when profiling (trace=True); harmless otherwise.
# ---------------------------------------------------------------------------
def _install_ntff_hook():
    try:
        import antenv  # noqa: F401

        if "antenv.axon_hooks" in sys.modules:
            return
        hooks_mod = types.ModuleType("antenv.axon_hooks")
        hooks_mod._hook = None

        def set_axon_ntff_profile_hook(h):
            hooks_mod._hook = h

        def get_axon_ntff_profile_hook():
            return hooks_mod._hook

        hooks_mod.set_axon_ntff_profile_hook = set_axon_ntff_profile_hook
        hooks_mod.get_axon_ntff_profile_hook = get_axon_ntff_profile_hook
        sys.modules["antenv.axon_hooks"] = hooks_mod
        import antenv as _a

        _a.axon_hooks = hooks_mod
        from trn_agent_boot.trn_boot import _ntff_profile_via_ctypes

        set_axon_ntff_profile_hook(
            _ntff_profile_via_ctypes("/opt/axon/libaxon_pjrt.so")
        )
    except Exception:
        pass


_install_ntff_hook()


def _install_verbose_cc_hook():
    """Wrap the PJRT->python compile callback so real tracebacks surface
    instead of an opaque 'CallFunctionObjArgs' error."""
    try:
        import traceback

        from concourse import bass2jax

        bass2jax.install_neuronx_cc_hook()
        import libneuronxla

        if getattr(libneuronxla, "_ant_verbose_wrap", False):
            return
        orig = libneuronxla.neuronx_cc

        def wrapped(*a, **k):
            try:
                return orig(*a, **k)
            except BaseException:
                traceback.print_exc()
                sys.stderr.flush()
                raise

        libneuronxla.neuronx_cc = wrapped
        libneuronxla._ant_verbose_wrap = True
        bass2jax.install_neuronx_cc_hook = lambda: None
    except Exception:
        pass


import bass_rust
import ml_dtypes
import concourse.bass as bass
import concourse.tile as tile
from concourse import mybir
from concourse.bass_utils import run_bass_kernel_spmd
from concourse.vector_clock import ScopedClock

BF16 = ml_dtypes.bfloat16

N_CORES = 8
B = 16  # batches total
S = 256  # seq len
D = 3800  # feature dim
H = 512  # hidden
C = 10  # classes

BLOC = B // N_CORES  # batches per core = 2
T = BLOC * S  # tokens per core = 512
DP = 3840  # D padded (+1 bias feature, up to 30*128)
KC = DP // 128  # 30 contraction chunks
ET = DP // 128  # 30 e-tiles of 128
PAIRS = KC // 2  # 15 DoubleRow chunk pairs
F32 = mybir.dt.float32
BF = mybir.dt.bfloat16
F8 = mybir.dt.float8e4
F8NP = mybir.dt.np(F8)  # ml_dtypes.float8_e4m3
# fp8 scale factors: weights are ~U(+-1/sqrt(3800)) which lands in e4m3's
# subnormal range, so weights are scaled up and the product scales are
# folded back out downstream (softmax scale / W1 scale).
WSCALE = 64.0  # on Wv
XASCALE = 16.0  # on abar (via the ones vector), so xa fits e4m3 nicely
SC_SCALE = 4096.0  # on M = Wq^T Wk / sqrt(D); scores' = 4096 * scores
H_SCALE = WSCALE * XASCALE  # hT' = 1024 * h


class SplitDrainTileContext(tile.TileContext):
    """This walrus build rejects >1 sync-wait on the tail Drain; split the
    global-clock waits across a chain of single-wait drain instructions."""

    MAXW = 1

    def _drain_and_barrier(self, tick_clock, wait_clock):
        nc = self.nc
        drain_inst = nc.sync.drain()
        wait_clock.add_sem_waits(
            drain_inst.ins, ScopedClock({None: tick_clock.global_clock})
        )
        si = drain_inst.ins.sync_info
        if si is not None and si.on_wait and len(si.on_wait) > self.MAXW:
            waits = list(si.on_wait)
            si.on_wait = waits[: self.MAXW]
            rest = waits[self.MAXW :]
            for i in range(0, len(rest), self.MAXW):
                extra = nc.sync.drain()
                extra.ins.sync_info = bass_rust.SyncInfo(
                    on_wait=rest[i : i + self.MAXW], on_update=[]
                )
        nc.all_engine_barrier()
        assert self.sems is not None
        popped = nc._tile_sem_poison_stack.pop()
        assert popped is self._sem_poison
        nc.clear_and_free_semaphores(list(self.sems.allocated().values()))
        nc.all_engine_barrier()


def _fix_excess_waits(nc, aux_sem, maxw=1):
    """Walrus in this image rejects instructions with more than ~1 sync
    wait. Compute-engine instructions: hoist extra waits onto same-engine
    no-ops inserted just before (sequencers execute in order). DMACopy:
    its waits live in the DGE queue descriptor, so an SP-side chain waits
    on all the original conditions, bumps `aux_sem`, and the descriptor
    waits on aux_sem alone."""
    aux_count = 0
    for f in nc.m.functions:
        for bb in f.blocks:
            insts = bb.instructions
            if not any(
                i.sync_info and i.sync_info.on_wait
                and len(i.sync_info.on_wait) > maxw
                for i in insts
            ):
                continue
            out = []
            for ins in insts:
                si = ins.sync_info
                nw = len(si.on_wait) if si and si.on_wait else 0
                if nw > maxw:
                    waits = list(si.on_wait)
                    if isinstance(ins, mybir.InstDMACopy):
                        for j, w in enumerate(waits):
                            nop = mybir.InstNoOp(name=f"{ins.name}-dw{j}")
                            nop.engine = mybir.EngineType.SP
                            nop.sync_info = bass_rust.SyncInfo(
                                on_wait=[w], on_update=[]
                            )
                            out.append(nop)
                        aux_count += 1
                        inc = mybir.InstNoOp(name=f"{ins.name}-dinc")
                        inc.engine = mybir.EngineType.SP
                        inc.sync_info = bass_rust.SyncInfo(
                            on_wait=[],
                            on_update=[
                                bass_rust.SyncUpdate(
                                    sync_type="semaphore",
                                    id=aux_sem.num,
                                    ant_name=aux_sem.name,
                                    update_mode="sem-add-imm",
                                    update_value=1,
                                    update_reg=None,
                                )
                            ],
                        )
                        out.append(inc)
                        si.on_wait = [
                            bass_rust.SyncWait(
                                sync_type="semaphore",
                                id=aux_sem.num,
                                ant_name=aux_sem.name,
                                wait_mode="sem-ge-imm",
                                wait_value=aux_count,
                                wait_reg=None,
                            )
                        ]
                    else:
                        keep = waits[-maxw:]
                        rest = waits[:-maxw]
                        for j, w in enumerate(rest):
                            nop = mybir.InstNoOp(name=f"{ins.name}-xw{j}")
                            nop.engine = ins.engine
                            nop.sync_info = bass_rust.SyncInfo(
                                on_wait=[w], on_update=[]
                            )
                            out.append(nop)
                        si.on_wait = keep
                out.append(ins)
            bb.instructions = out
    if aux_count:
        # reset aux sem at the very end so a re-executed NEFF starts clean
        f = nc.m.functions[0]
        bb = list(f.blocks)[-1]
        rst = mybir.InstNoOp(name="auxwait-reset")
        rst.engine = mybir.EngineType.SP
        rst.sync_info = bass_rust.SyncInfo(
            on_wait=[],
            on_update=[
                bass_rust.SyncUpdate(
                    sync_type="semaphore",
                    id=aux_sem.num,
                    ant_name=aux_sem.name,
                    update_mode="sem-sub-imm",
                    update_value=aux_count,
                    update_reg=None,
                )
            ],
        )
        il = bb.instructions
        il.append(rst)
        bb.instructions = il


def build_kernel() -> bass.Bass:
    nc = bass.Bass()

    x_d = nc.declare_dram_parameter("x8", [128, PAIRS, 2, T], F8, isOutput=False)
    xtok_d = nc.declare_dram_parameter("xtok", [128, 4, DP], BF, isOutput=False)
    m8_d = nc.declare_dram_parameter("m8", [ET, 128, PAIRS, 2, 128], F8,
                                     isOutput=False)
    wv_d = nc.declare_dram_parameter("wv", [ET, 128, KC, 128], F8, isOutput=False)
    w1_d = nc.declare_dram_parameter("w1", [128, KC, H], BF, isOutput=False)
    w2_d = nc.declare_dram_parameter("w2", [128, 5, H], F32, isOutput=False)
    w3_d = nc.declare_dram_parameter("w3", [128, 5, C], F32, isOutput=False)
    e0b_d = nc.declare_dram_parameter("e0b", [128, BLOC], F32, isOutput=False)
    out_d = nc.declare_dram_parameter("outT", [C, BLOC], F32, isOutput=True)

    aux_sem = nc.alloc_semaphore("auxwait")
    with SplitDrainTileContext(nc) as tc:
        with tc.tile_pool(name="persist", bufs=1) as persist:
            _emit(nc, tc, persist, x_d, xtok_d, m8_d, wv_d, w1_d, w2_d,
                  w3_d, e0b_d, out_d)
    _fix_excess_waits(nc, aux_sem)
    return nc


def _emit(nc, tc, persist, x_d, xtok_d, m8_d, wv_d, w1_d, w2_d, w3_d,
          e0b_d, out_d):
    # ------------------ persistent tiles ------------------
    # x8 split per DoubleRow pair so early matmuls only wait on their own
    # slice's DMA (Tile dependencies are whole-tile).
    x8c = [persist.tile([128, 2, T], F8, name=f"x8{p}", tag=f"x8{p}")
           for p in range(PAIRS)]
    ones_s = persist.tile([128, 1], BF)
    nc.vector.memset(ones_s[:], XASCALE / S)
    a_bar2 = persist.tile([128, 4, BLOC], BF)
    nc.vector.memset(a_bar2[:], 0.0)
    x_tok = persist.tile([128, 4, DP], BF)
    xaT = persist.tile([128, KC, BLOC], F8)
    hT = persist.tile([128, KC, BLOC], BF)
    # t1 = (M8^T x8): fp8, [d2 within tile, d2-tile, token]
    t1_sb = persist.tile([128, KC, T], F8)

    # MLP weights: tiles up-front, DMAs issued a few iterations into
    # phase 1 so they overlap compute instead of the critical startup
    mlpw = tc.alloc_tile_pool(name="mlpw", bufs=1)
    w1_t = mlpw.tile([128, KC, H], BF)
    w2_t = mlpw.tile([128, 5, H], F32)
    w3_t = mlpw.tile([128, 5, C], F32)

    # ---- phase 1a: t1 = M^T x  (scores = x M x^T = t1^T x, M = Wq^T Wk) ----
    DR = mybir.MatmulPerfMode.DoubleRow
    with tc.tile_pool(name="psum_sc", bufs=1, space="PSUM") as psum_sc:
        ps = [
            psum_sc.tile([128, S], F32, name=f"scores{i}", tag=f"scores{i}")
            for i in range(4)  # index = 2*b + it
        ]
        with (
            tc.tile_pool(name="mpool", bufs=1) as mpool,
            tc.tile_pool(name="psum_kq", bufs=1, space="PSUM") as psum_kq,
        ):
            for d2t in range(ET):
                m_t = mpool.tile([128, PAIRS, 2, 128], F8, tag="m8", bufs=3)
                if d2t == 0:
                    # interleave the first M block's pairs with the x8 loads
                    # so the very first matmul only waits on ~160KB
                    nc.sync.dma_start(m_t[:, 0], m8_d[0, :, 0])
                    nc.sync.dma_start(x8c[0][:], x_d[:, 0])
                    for p in range(1, PAIRS):
                        nc.sync.dma_start(m_t[:, p], m8_d[0, :, p])
                        nc.sync.dma_start(x8c[p][:], x_d[:, p])
                else:
                    nc.sync.dma_start(m_t[:], m8_d[d2t])
                if d2t == 3:
                    for tt in range(4):
                        nc.sync.dma_start(x_tok[:, tt, :], xtok_d[:, tt, :])
                    nc.sync.dma_start(w2_t[:], w2_d[:])
                    nc.sync.dma_start(w3_t[:], w3_d[:])
                if 6 <= d2t < 6 + KC // 2:
                    # spread the 30 W1 chunk loads over phase-1 iterations
                    kc0 = 2 * (d2t - 6)
                    nc.sync.dma_start(w1_t[:, kc0 : kc0 + 2, :],
                                      w1_d[:, kc0 : kc0 + 2, :])

                pt = psum_kq.tile([128, T], F32, tag="pt", bufs=2)
                for p in range(PAIRS):
                    nc.tensor.matmul(
                        pt[:], m_t[:, p], x8c[p][:],
                        start=(p == 0), stop=(p == PAIRS - 1),
                        perf_mode=DR,
                    )
                nc.vector.tensor_copy(t1_sb[:, d2t, :], pt[:])

            # ---- phase 1b: scores'[i, j] = sum_d2 t1[d2, i] x8[d2, j] ----
            for b in range(BLOC):
                for it in range(2):
                    i0 = b * S + it * 128
                    for p in range(PAIRS):
                        nc.tensor.matmul(
                            ps[2 * b + it][:],
                            t1_sb[:, 2 * p : 2 * p + 2, i0 : i0 + 128],
                            x8c[p][:, :, b * S : (b + 1) * S],
                            start=(p == 0), stop=(p == PAIRS - 1),
                            perf_mode=DR,
                        )

        # ------------- phase 2: softmax + abar (column means) -------------
        with (
            tc.tile_pool(name="smx", bufs=1) as smx,
            tc.tile_pool(name="psum_ab", bufs=1, space="PSUM") as psum_ab,
        ):
            pab = [
                psum_ab.tile([128, 1], F32, name=f"pab{i}", tag=f"pab{i}")
                for i in range(4)  # index = 2*b + jc
            ]
            for b in range(BLOC):
                for it in range(2):
                    p = ps[2 * b + it]
                    mx = smx.tile([128, 1], F32, tag="mx", bufs=2)
                    nc.vector.reduce_max(
                        out=mx[:], in_=p[:], axis=mybir.AxisListType.X
                    )
                    negm = smx.tile([128, 1], F32, tag="negm", bufs=2)
                    nc.vector.tensor_scalar_mul(negm[:], mx[:], -1.0 / SC_SCALE)
                    pexp = smx.tile([128, S], F32, tag="pexp", bufs=2)
                    sm = smx.tile([128, 1], F32, tag="sm", bufs=2)
                    nc.scalar.activation(
                        pexp[:], p[:], mybir.ActivationFunctionType.Exp,
                        bias=negm[:], scale=1.0 / SC_SCALE, accum_out=sm[:],
                    )
                    rin = smx.tile([128, 1], F32, tag="rin", bufs=2)
                    nc.vector.reciprocal(rin[:], sm[:])
                    attn_b = smx.tile([128, S], BF, tag="attn", bufs=2)
                    nc.vector.tensor_scalar_mul(attn_b[:], pexp[:], rin[:])
                    for jc in range(2):
                        nc.tensor.matmul(
                            pab[2 * b + jc][:],
                            attn_b[:, jc * 128 : (jc + 1) * 128],
                            ones_s[:],
                            start=(it == 0), stop=(it == 1),
                            skip_group_check=True,
                        )
            for b in range(BLOC):
                for jc in range(2):
                    nc.vector.tensor_copy(
                        a_bar2[:, 2 * b + jc, b : b + 1], pab[2 * b + jc][:]
                    )

    # ------------------ phase 3: xa = abar @ x ------------------
    with tc.tile_pool(name="psum_xa", bufs=1, space="PSUM") as psum_xa:
        for dt in range(KC):
            pxa = psum_xa.tile([128, BLOC], F32, tag="pxa", bufs=2)
            for tt in range(4):
                nc.tensor.matmul(
                    pxa[:],
                    x_tok[:, tt, dt * 128 : (dt + 1) * 128],
                    a_bar2[:, tt, :],
                    start=(tt == 0), stop=(tt == 3),
                )
            nc.vector.tensor_copy(xaT[:, dt, :], pxa[:])

    # ------------------ phase 4: hT = Wv^T-tiles @ xaT ------------------
    with (
        tc.tile_pool(name="wv", bufs=1) as wv_pool,
        tc.tile_pool(name="psum_h", bufs=1, space="PSUM") as psum_h,
    ):
        for et in range(ET):
            wv_t = wv_pool.tile([128, KC, 128], F8, tag="wv", bufs=16)
            nc.sync.dma_start(wv_t[:], wv_d[et])
            ph = psum_h.tile([128, BLOC], F32, tag="ph", bufs=2)
            for kc in range(KC):
                nc.tensor.matmul(
                    ph[:], wv_t[:, kc, :], xaT[:, kc, :],
                    start=(kc == 0), stop=(kc == KC - 1),
                )
            nc.vector.tensor_copy(hT[:, et, :], ph[:])

    # ------------------ phase 5: MLP (fp32) ------------------
    with (
        tc.tile_pool(name="mlph", bufs=1) as mlph,
        tc.tile_pool(name="psum_m", bufs=1, space="PSUM") as psum_m,
    ):
        h1T = mlph.tile([128, 5, BLOC], F32)
        nc.sync.dma_start(h1T[:, 4, :], e0b_d[:])
        for ot in range(4):
            pm = psum_m.tile([128, BLOC], F32, tag="pm1", bufs=2)
            for kc in range(KC):
                nc.tensor.matmul(
                    pm[:],
                    w1_t[:, kc, ot * 128 : (ot + 1) * 128],
                    hT[:, kc, :],
                    start=(kc == 0), stop=(kc == KC - 1),
                )
            nc.scalar.activation(
                h1T[:, ot, :], pm[:], mybir.ActivationFunctionType.Sigmoid
            )

        h2T = mlph.tile([128, 5, BLOC], F32)
        nc.sync.dma_start(h2T[:, 4, :], e0b_d[:])
        for ot in range(4):
            pm = psum_m.tile([128, BLOC], F32, tag="pm2", bufs=2)
            for oc in range(5):
                nc.tensor.matmul(
                    pm[:],
                    w2_t[:, oc, ot * 128 : (ot + 1) * 128],
                    h1T[:, oc, :],
                    start=(oc == 0), stop=(oc == 4),
                )
            nc.scalar.activation(
                h2T[:, ot, :], pm[:], mybir.ActivationFunctionType.Sigmoid
            )

        pm3 = psum_m.tile([C, BLOC], F32, tag="pm3")
        for oc in range(5):
            nc.tensor.matmul(
                pm3[:],
                w3_t[:, oc, :],
                h2T[:, oc, :],
                start=(oc == 0), stop=(oc == 4),
            )
        out_sb = mlph.tile([C, BLOC], F32)
        nc.vector.tensor_copy(out_sb[:], pm3[:])
        nc.sync.dma_start(out_d[:], out_sb[:])
    mlpw.release()


# ---------------------------------------------------------------------------
# Host-side packing
# ---------------------------------------------------------------------------
def _pack_m8(Wq, bq, Wk, bk):
    """M = Wq'^T Wk' / sqrt(D), where W' carries its bias in column d=3800.
    scores = x' M x'^T reproduces q @ k.T / sqrt(D) exactly (the unit bias
    feature of x' supplies the bias cross terms). Scaled by SC_SCALE for
    e4m3 range, DoubleRow-interleaved to [ET, 128, PAIRS, 2, 128]:
    A[d2t, d1p, p, ko, d2p] = SC_SCALE * M[(2p+ko)*128+d1p, d2t*128+d2p]."""
    Wqp = np.zeros((D, DP), dtype=np.float32)
    Wqp[:, :D] = Wq
    Wqp[:, D] = bq
    Wkp = np.zeros((D, DP), dtype=np.float32)
    Wkp[:, :D] = Wk
    Wkp[:, D] = bk
    M = (Wqp.T @ Wkp) * np.float32(SC_SCALE / np.sqrt(np.float64(D)))
    A = M.reshape(PAIRS, 2, 128, ET, 128).transpose(3, 2, 0, 1, 4)
    return np.ascontiguousarray(A, dtype=F8NP)


def _pack_wv8(W, bias):
    """W [D, D], bias [D] -> [ET, 128, KC, 128] e4m3 with
    A[et, dp, kc, ep] = WSCALE * Wp[et*128+ep, kc*128+dp]; bias in column
    d=3800; unit row at e=3800 propagates the bias feature into hT."""
    Wp = np.zeros((DP, DP), dtype=np.float32)
    Wp[:D, :D] = W * WSCALE
    Wp[:D, D] = bias * WSCALE
    Wp[D, D] = WSCALE
    A = Wp.reshape(ET, 128, KC, 128).transpose(0, 3, 2, 1)
    return np.ascontiguousarray(A, dtype=F8NP)


def _pack_x8(xc):
    """xc [BLOC, S, D] -> [128, PAIRS, 2, T] e4m3, bias row d=3800 = 1."""
    xt = np.zeros((DP, T), dtype=np.float32)
    xt[:D, :] = xc.reshape(T, D).T
    xt[D, :] = 1.0
    A = xt.reshape(PAIRS, 2, 128, T).transpose(2, 0, 1, 3)
    return np.ascontiguousarray(A, dtype=F8NP)


def _pack_xtok(xc):
    """xc [BLOC, S, D] -> [128, 4, DP] bf16 (token partition), col d=3800 = 1."""
    xp = np.zeros((T, DP), dtype=np.float32)
    xp[:, :D] = xc.reshape(T, D)
    xp[:, D] = 1.0
    A = xp.reshape(4, 128, DP).transpose(1, 0, 2)
    return np.ascontiguousarray(A, dtype=BF16)


def _pack_w1(W1, b1):
    """W1 [H, D] -> [128, KC, H] bf16: A[dp, kc, o] = W1p[o, kc*128+dp] with
    the hT scale (1/H_SCALE) folded in; b1 in column d=3800 (hT[3800] ==
    H_SCALE)."""
    W1p = np.zeros((H, DP), dtype=np.float32)
    W1p[:, :D] = W1 / np.float32(H_SCALE)
    W1p[:, D] = b1 / np.float32(H_SCALE)
    A = W1p.T.reshape(KC, 128, H).transpose(1, 0, 2)
    return np.ascontiguousarray(A, dtype=BF16)


def _pack_w2(W2, b2):
    A = np.zeros((128, 5, H), dtype=np.float32)
    A[:, :4, :] = W2.T.reshape(4, 128, H).transpose(1, 0, 2)
    A[0, 4, :] = b2
    return np.ascontiguousarray(A)


def _pack_w3(W3, b3):
    A = np.zeros((128, 5, C), dtype=np.float32)
    A[:, :4, :] = W3.T.reshape(4, 128, C).transpose(1, 0, 2)
    A[0, 4, :] = b3
    return np.ascontiguousarray(A)


_NC_CACHE = {}


def _get_nc():
    if "nc" not in _NC_CACHE:
        _NC_CACHE["nc"] = build_kernel()
    return _NC_CACHE["nc"]


def kernel(x, Wk, bk, Wq, bq, Wv, bv, W1, b1, W2, b2, W3, b3, _trace=False):
    x = np.asarray(x, dtype=np.float32)

    m8_p = _pack_m8(
        np.asarray(Wq, np.float32), np.asarray(bq, np.float32),
        np.asarray(Wk, np.float32), np.asarray(bk, np.float32),
    )
    wv_p = _pack_wv8(np.asarray(Wv, np.float32), np.asarray(bv, np.float32))
    w1_p = _pack_w1(np.asarray(W1, np.float32), np.asarray(b1, np.float32))
    w2_p = _pack_w2(np.asarray(W2, np.float32), np.asarray(b2, np.float32))
    w3_p = _pack_w3(np.asarray(W3, np.float32), np.asarray(b3, np.float32))
    e0b = np.zeros((128, BLOC), dtype=np.float32)
    e0b[0, :] = 1.0

    in_maps = []
    for c in range(N_CORES):
        xc = x[c * BLOC : (c + 1) * BLOC]
        in_maps.append(
            {
                "x8": _pack_x8(xc),
                "xtok": _pack_xtok(xc),
                "m8": m8_p,
                "wv": wv_p,
                "w1": w1_p,
                "w2": w2_p,
                "w3": w3_p,
                "e0b": e0b,
            }
        )

    nc = _get_nc()
    _install_verbose_cc_hook()
    res = run_bass_kernel_spmd(nc, in_maps, list(range(N_CORES)), trace=_trace)
    out = np.zeros((B, C), dtype=np.float32)
    for c in range(N_CORES):
        out[c * BLOC : (c + 1) * BLOC] = res.results[c]["outT"].T
    if _trace:
        return out, res
    return out



# revision 10
# speedup vs baseline: 1.4868x; 1.4868x over previous
"""Trainium2 Bass kernel for AttentionMLP (nn_AttentionMLP_72997264163220).

Reference computation:
  k/q/v = x @ W{k,q,v}.T + b      (D=3800 -> D)
  scores = q @ k.T / sqrt(D); attn = softmax(scores, -1)
  attended = attn @ v; h = attended.mean(seq)
  h = sigmoid(h @ W1.T + b1); h = sigmoid(h @ W2.T + b2); out = h @ W3.T + b3

Algebraic simplifications (all host-side folds):
  1. scores = x' M x'^T with M = Wq'^T Wk' / sqrt(D) precomputed on the
     host (biases ride along in a unit feature at d=3800). q and k are
     never materialized.
  2. The mean over the sequence commutes with the attention matmul, the
     (linear) v projection AND the first MLP layer:
       h1pre = mean_i(attn) @ v @ W1^T + b1 = abar @ x' @ Z^T
     with Z = W1' @ Wv' [H, D'] precomputed on the host. So v, Wv and W1
     never appear on-device: one G = x' @ Z^T [T, H] matmul (independent
     of attention, overlapped with softmax) plus a tiny abar @ G.

Sharding: data-parallel over batch. 16 batches -> 8 cores x 2 batches
(512 tokens per core). All weights replicated, host pre-packed / cast.
Big matmuls in fp8 DoubleRow (fp32 PSUM accumulate); softmax and the
tail MLP in fp32.

Device dataflow per core (SBUF partition dim always first; D padded to
3840 = 30*128 with the bias feature at d=3800):
  x8    [128, 15, 2, 512] fp8  x'^T DR-paired (d1p, pair, ko, token)
  per d2-tile (30): t1[d2t] = M^T x'  (15 DR matmuls, N=512)
  score MMs interleaved 2 chunks behind: ps[2b+it] += t1_pair^T x8_pair
  softmax without max-subtraction (scores are O(1)): pexp = exp(ps/SC),
  r = 1/(S*rowsum); abar = pexp^T r via matmul (attn never normalized)
  G = x'^T-pairs @ Z^T  (60 DR matmuls, runs on PE while softmax runs
  on ACT) -> G_sb bf16 [128 t, 4, 512]
  h1T[128h,4ot,2b] = sigmoid(G^T abar); fp32 MLP tail as before.
"""

import sys
import types

import numpy as np

if "/opt/trn_rl_repo" not in sys.path:
    sys.path.insert(0, "/opt/trn_rl_repo")


# ---------------------------------------------------------------------------
# NTFF profile hook shim (antenv.axon_hooks is absent in this image). Needed
# only when profiling (trace=True); harmless otherwise.
# ---------------------------------------------------------------------------
def _install_ntff_hook():
    try:
        import antenv  # noqa: F401

        if "antenv.axon_hooks" in sys.modules:
            return
        hooks_mod = types.ModuleType("antenv.axon_hooks")
        hooks_mod._hook = None

        def set_axon_ntff_profile_hook(h):
            hooks_mod._hook = h

        def get_axon_ntff_profile_hook():
            return hooks_mod._hook

        hooks_mod.set_axon_ntff_profile_hook = set_axon_ntff_profile_hook
        hooks_mod.get_axon_ntff_profile_hook = get_axon_ntff_profile_hook
        sys.modules["antenv.axon_hooks"] = hooks_mod
        import antenv as _a

        _a.axon_hooks = hooks_mod
        from trn_agent_boot.trn_boot import _ntff_profile_via_ctypes

        set_axon_ntff_profile_hook(
            _ntff_profile_via_ctypes("/opt/axon/libaxon_pjrt.so")
        )
    except Exception:
        pass


_install_ntff_hook()


def _install_verbose_cc_hook():
    """Wrap the PJRT->python compile callback so real tracebacks surface
    instead of an opaque 'CallFunctionObjArgs' error."""
    try:
        import traceback

        from concourse import bass2jax

        bass2jax.install_neuronx_cc_hook()
        import libneuronxla

        if getattr(libneuronxla, "_ant_verbose_wrap", False):
            return
        orig = libneuronxla.neuronx_cc

        def wrapped(*a, **k):
            try:
                return orig(*a, **k)
            except BaseException:
                traceback.print_exc()
                sys.stderr.flush()
                raise

        libneuronxla.neuronx_cc = wrapped
        libneuronxla._ant_verbose_wrap = True
        bass2jax.install_neuronx_cc_hook = lambda: None
    except Exception:
        pass


import bass_rust
import ml_dtypes
import concourse.bass as bass
import concourse.tile as tile
from concourse import mybir
from concourse.bass_utils import run_bass_kernel_spmd
from concourse.vector_clock import ScopedClock

BF16 = ml_dtypes.bfloat16

N_CORES = 8
B = 16  # batches total
S = 256  # seq len
D = 3800  # feature dim
H = 512  # hidden
C = 10  # classes

BLOC = B // N_CORES  # batches per core = 2
T = BLOC * S  # tokens per core = 512
DP = 3840  # D padded (+1 bias feature, up to 30*128)
KC = DP // 128  # 30 contraction chunks
ET = DP // 128  # 30 e-tiles of 128
PAIRS = KC // 2  # 15 DoubleRow chunk pairs
F32 = mybir.dt.float32
BF = mybir.dt.bfloat16
F8 = mybir.dt.float8e4
F8NP = mybir.dt.np(F8)  # ml_dtypes.float8_e4m3
# fp8 scale factors: weights are ~U(+-1/sqrt(3800)) which lands in e4m3's
# subnormal range, so weights are scaled up and the product scales are
# folded back out downstream (softmax scale / sigmoid scale).
SC_SCALE = 4096.0  # on M = Wq^T Wk / sqrt(D); scores' = 4096 * scores
ZSC = 32.0  # on Z = W1' @ Wv'; h1pre' = ZSC * h1pre


class SplitDrainTileContext(tile.TileContext):
    """This walrus build rejects >1 sync-wait on the tail Drain; split the
    global-clock waits across a chain of single-wait drain instructions."""

    MAXW = 1

    def _drain_and_barrier(self, tick_clock, wait_clock):
        nc = self.nc
        drain_inst = nc.sync.drain()
        wait_clock.add_sem_waits(
            drain_inst.ins, ScopedClock({None: tick_clock.global_clock})
        )
        si = drain_inst.ins.sync_info
        if si is not None and si.on_wait and len(si.on_wait) > self.MAXW:
            waits = list(si.on_wait)
            si.on_wait = waits[: self.MAXW]
            rest = waits[self.MAXW :]
            for i in range(0, len(rest), self.MAXW):
                extra = nc.sync.drain()
                extra.ins.sync_info = bass_rust.SyncInfo(
                    on_wait=rest[i : i + self.MAXW], on_update=[]
                )
        nc.all_engine_barrier()
        assert self.sems is not None
        popped = nc._tile_sem_poison_stack.pop()
        assert popped is self._sem_poison
        nc.clear_and_free_semaphores(list(self.sems.allocated().values()))
        nc.all_engine_barrier()


def _fix_excess_waits(nc, aux_sem, maxw=1):
    """Walrus in this image rejects instructions with more than ~1 sync
    wait. Compute-engine instructions: hoist extra waits onto same-engine
    no-ops inserted just before (sequencers execute in order). DMACopy:
    its waits live in the DGE queue descriptor, so an SP-side chain waits
    on all the original conditions, bumps `aux_sem`, and the descriptor
    waits on aux_sem alone."""
    aux_count = 0
    for f in nc.m.functions:
        for bb in f.blocks:
            insts = bb.instructions
            if not any(
                i.sync_info and i.sync_info.on_wait
                and len(i.sync_info.on_wait) > maxw
                for i in insts
            ):
                continue
            out = []
            for ins in insts:
                si = ins.sync_info
                nw = len(si.on_wait) if si and si.on_wait else 0
                if nw > maxw:
                    waits = list(si.on_wait)
                    if isinstance(ins, mybir.InstDMACopy):
                        for j, w in enumerate(waits):
                            nop = mybir.InstNoOp(name=f"{ins.name}-dw{j}")
                            nop.engine = mybir.EngineType.SP
                            nop.sync_info = bass_rust.SyncInfo(
                                on_wait=[w], on_update=[]
                            )
                            out.append(nop)
                        aux_count += 1
                        inc = mybir.InstNoOp(name=f"{ins.name}-dinc")
                        inc.engine = mybir.EngineType.SP
                        inc.sync_info = bass_rust.SyncInfo(
                            on_wait=[],
                            on_update=[
                                bass_rust.SyncUpdate(
                                    sync_type="semaphore",
                                    id=aux_sem.num,
                                    ant_name=aux_sem.name,
                                    update_mode="sem-add-imm",
                                    update_value=1,
                                    update_reg=None,
                                )
                            ],
                        )
                        out.append(inc)
                        si.on_wait = [
                            bass_rust.SyncWait(
                                sync_type="semaphore",
                                id=aux_sem.num,
                                ant_name=aux_sem.name,
                                wait_mode="sem-ge-imm",
                                wait_value=aux_count,
                                wait_reg=None,
                            )
                        ]
                    else:
                        keep = waits[-maxw:]
                        rest = waits[:-maxw]
                        for j, w in enumerate(rest):
                            nop = mybir.InstNoOp(name=f"{ins.name}-xw{j}")
                            nop.engine = ins.engine
                            nop.sync_info = bass_rust.SyncInfo(
                                on_wait=[w], on_update=[]
                            )
                            out.append(nop)
                        si.on_wait = keep
                out.append(ins)
            bb.instructions = out
    if aux_count:
        # reset aux sem at the very end so a re-executed NEFF starts clean
        f = nc.m.functions[0]
        bb = list(f.blocks)[-1]
        rst = mybir.InstNoOp(name="auxwait-reset")
        rst.engine = mybir.EngineType.SP
        rst.sync_info = bass_rust.SyncInfo(
            on_wait=[],
            on_update=[
                bass_rust.SyncUpdate(
                    sync_type="semaphore",
                    id=aux_sem.num,
                    ant_name=aux_sem.name,
                    update_mode="sem-sub-imm",
                    update_value=aux_count,
                    update_reg=None,
                )
            ],
        )
        il = bb.instructions
        il.append(rst)
        bb.instructions = il


def build_kernel() -> bass.Bass:
    nc = bass.Bass()

    x_d = nc.declare_dram_parameter("x8", [128, PAIRS, 2, T], F8, isOutput=False)
    m8_d = nc.declare_dram_parameter("m8", [ET, 128, PAIRS, 2, 128], F8,
                                     isOutput=False)
    zt_d = nc.declare_dram_parameter("zt", [128, PAIRS, 2, H], F8, isOutput=False)
    w2_d = nc.declare_dram_parameter("w2", [128, 5, H], F32, isOutput=False)
    w3_d = nc.declare_dram_parameter("w3", [128, 5, C], F32, isOutput=False)
    e0b_d = nc.declare_dram_parameter("e0b", [128, BLOC], F32, isOutput=False)
    out_d = nc.declare_dram_parameter("outT", [C, BLOC], F32, isOutput=True)

    aux_sem = nc.alloc_semaphore("auxwait")
    with SplitDrainTileContext(nc) as tc:
        with tc.tile_pool(name="persist", bufs=1) as persist:
            _emit(nc, tc, persist, x_d, m8_d, zt_d, w2_d, w3_d, e0b_d, out_d)
    _fix_excess_waits(nc, aux_sem)
    return nc


def _emit(nc, tc, persist, x_d, m8_d, zt_d, w2_d, w3_d, e0b_d, out_d):
    # ------------------ persistent tiles ------------------
    # x8 split per DoubleRow pair so early matmuls only wait on their own
    # slice's DMA (Tile dependencies are whole-tile).
    x8c = [persist.tile([128, 2, T], F8, name=f"x8{p}", tag=f"x8{p}")
           for p in range(PAIRS)]
    a_bar2 = persist.tile([128, 4, BLOC], BF)
    nc.vector.memset(a_bar2[:], 0.0)
    # t1 = (M8^T x8): fp8, [d2 within tile, d2-tile, token]
    t1_sb = persist.tile([128, KC, T], F8)
    zt_t = persist.tile([128, PAIRS, 2, H], F8)
    g_sb = persist.tile([128, 4, H], BF)

    # MLP weights: tiles up-front, DMAs issued a few iterations into
    # phase 1 so they overlap compute instead of the critical startup
    mlpw = tc.alloc_tile_pool(name="mlpw", bufs=1)
    w2_t = mlpw.tile([128, 5, H], F32)
    w3_t = mlpw.tile([128, 5, C], F32)

    # ---- phase 1a: t1 = M^T x  (scores = x M x^T = t1^T x, M = Wq^T Wk) ----
    # Score matmuls (phase 1b) are interleaved into the same loop with a
    # 2-chunk lag so the PE never waits on the DVE psum->sbuf cast.
    DR = mybir.MatmulPerfMode.DoubleRow
    with tc.tile_pool(name="psum_sc", bufs=1, space="PSUM") as psum_sc:
        ps = [
            psum_sc.tile([128, S], F32, name=f"scores{i}", tag=f"scores{i}")
            for i in range(4)  # index = 2*b + it
        ]

        def emit_scores_pair(p):
            for b in range(BLOC):
                for it in range(2):
                    i0 = b * S + it * 128
                    nc.tensor.matmul(
                        ps[2 * b + it][:],
                        t1_sb[:, 2 * p : 2 * p + 2, i0 : i0 + 128],
                        x8c[p][:, :, b * S : (b + 1) * S],
                        start=(p == 0), stop=(p == PAIRS - 1),
                        perf_mode=DR,
                        skip_group_check=True,
                    )

        with (
            tc.tile_pool(name="mpool", bufs=1) as mpool,
            tc.tile_pool(name="psum_kq", bufs=1, space="PSUM") as psum_kq,
        ):
            for d2t in range(ET):
                m_t = mpool.tile([128, PAIRS, 2, 128], F8, tag="m8", bufs=3)
                if d2t == 0:
                    # interleave the first M block's pairs with the x8 loads
                    # so the very first matmul only waits on ~160KB
                    nc.sync.dma_start(m_t[:, 0], m8_d[0, :, 0])
                    nc.sync.dma_start(x8c[0][:], x_d[:, 0])
                    for p in range(1, PAIRS):
                        nc.sync.dma_start(m_t[:, p], m8_d[0, :, p])
                        nc.sync.dma_start(x8c[p][:], x_d[:, p])
                else:
                    nc.sync.dma_start(m_t[:], m8_d[d2t])
                if d2t == 3:
                    nc.sync.dma_start(zt_t[:], zt_d[:])
                if d2t == 5:
                    nc.sync.dma_start(w2_t[:], w2_d[:])
                    nc.sync.dma_start(w3_t[:], w3_d[:])

                pt = psum_kq.tile([128, T], F32, tag="pt", bufs=2)
                for p in range(PAIRS):
                    nc.tensor.matmul(
                        pt[:], m_t[:, p], x8c[p][:],
                        start=(p == 0), stop=(p == PAIRS - 1),
                        perf_mode=DR,
                    )
                nc.vector.tensor_copy(t1_sb[:, d2t, :], pt[:])
                if d2t >= 3 and d2t % 2 == 1:
                    emit_scores_pair((d2t - 3) // 2)
            emit_scores_pair(PAIRS - 1)

        # ---- phase 2: G = x' @ Z^T on PE  ||  softmax + abar on ACT ----
        # G is independent of the attention, so the PE chews through it
        # while the scalar engine runs the softmax chain.
        with (
            tc.tile_pool(name="smx", bufs=1) as smx,
            tc.tile_pool(name="psum_g", bufs=1, space="PSUM") as psum_g,
            tc.tile_pool(name="psum_ab", bufs=1, space="PSUM") as psum_ab,
        ):
            # softmax chain first in program order on ACT; its matmuls are
            # emitted after G's so the PE is never idle waiting on ACT.
            pexps = []
            rs = []
            for b in range(BLOC):
                for it in range(2):
                    pexp = smx.tile([128, S], F32, tag=f"pexp{2*b+it}")
                    sm = smx.tile([128, 1], F32, tag="sm", bufs=2)
                    # no max-subtraction: scores are O(1) by construction
                    nc.scalar.activation(
                        pexp[:], ps[2 * b + it][:],
                        mybir.ActivationFunctionType.Exp,
                        scale=1.0 / SC_SCALE, accum_out=sm[:],
                    )
                    r = smx.tile([128, 1], F32, tag="r", bufs=4)
                    # r = 1/rowsum; the 1/S mean is folded into the sigmoid
                    # scale of the first MLP layer
                    nc.vector.reciprocal(r[:], sm[:])
                    pexps.append(pexp)
                    rs.append(r)

            for tcn in range(4):
                pg = psum_g.tile([128, H], F32, tag="pg", bufs=2)
                for p in range(PAIRS):
                    nc.tensor.matmul(
                        pg[:],
                        x8c[p][:, :, tcn * 128 : (tcn + 1) * 128],
                        zt_t[:, p],
                        start=(p == 0), stop=(p == PAIRS - 1),
                        perf_mode=DR,
                    )
                nc.vector.tensor_copy(g_sb[:, tcn, :], pg[:])

            # abar[j] = sum_i pexp[i, j] * r[i]  (fp32 matmul, N=1)
            pab = psum_ab.tile([128, 4], F32, name="pab")
            for b in range(BLOC):
                for it in range(2):
                    for jc in range(2):
                        nc.tensor.matmul(
                            pab[:, 2 * b + jc : 2 * b + jc + 1],
                            pexps[2 * b + it][:, jc * 128 : (jc + 1) * 128],
                            rs[2 * b + it][:],
                            start=(it == 0), stop=(it == 1),
                            skip_group_check=True,
                        )
            for b in range(BLOC):
                for jc in range(2):
                    nc.vector.tensor_copy(
                        a_bar2[:, 2 * b + jc, b : b + 1],
                        pab[:, 2 * b + jc : 2 * b + jc + 1],
                    )

    # ------------------ phase 3: MLP (fp32) ------------------
    with (
        tc.tile_pool(name="mlph", bufs=1) as mlph,
        tc.tile_pool(name="psum_m", bufs=1, space="PSUM") as psum_m,
    ):
        h1T = mlph.tile([128, 5, BLOC], F32)
        nc.sync.dma_start(h1T[:, 4, :], e0b_d[:])
        for ot in range(4):
            pm = psum_m.tile([128, BLOC], F32, tag="pm1", bufs=2)
            for tt in range(4):
                nc.tensor.matmul(
                    pm[:],
                    g_sb[:, tt, ot * 128 : (ot + 1) * 128],
                    a_bar2[:, tt, :],
                    start=(tt == 0), stop=(tt == 3),
                )
            nc.scalar.activation(
                h1T[:, ot, :], pm[:], mybir.ActivationFunctionType.Sigmoid,
                scale=1.0 / (ZSC * S),
            )

        h2T = mlph.tile([128, 5, BLOC], F32)
        nc.sync.dma_start(h2T[:, 4, :], e0b_d[:])
        for ot in range(4):
            pm = psum_m.tile([128, BLOC], F32, tag="pm2", bufs=2)
            for oc in range(5):
                nc.tensor.matmul(
                    pm[:],
                    w2_t[:, oc, ot * 128 : (ot + 1) * 128],
                    h1T[:, oc, :],
                    start=(oc == 0), stop=(oc == 4),
                )
            nc.scalar.activation(
                h2T[:, ot, :], pm[:], mybir.ActivationFunctionType.Sigmoid
            )

        pm3 = psum_m.tile([C, BLOC], F32, tag="pm3")
        for oc in range(5):
            nc.tensor.matmul(
                pm3[:],
                w3_t[:, oc, :],
                h2T[:, oc, :],
                start=(oc == 0), stop=(oc == 4),
            )
        out_sb = mlph.tile([C, BLOC], F32)
        nc.vector.tensor_copy(out_sb[:], pm3[:])
        nc.sync.dma_start(out_d[:], out_sb[:])
    mlpw.release()


# ---------------------------------------------------------------------------
# Host-side packing
# ---------------------------------------------------------------------------
def _pack_m8(Wq, bq, Wk, bk):
    """M = Wq'^T Wk' / sqrt(D), where W' carries its bias in column d=3800.
    scores = x' M x'^T reproduces q @ k.T / sqrt(D) exactly (the unit bias
    feature of x' supplies the bias cross terms). Scaled by SC_SCALE for
    e4m3 range, DoubleRow-interleaved to [ET, 128, PAIRS, 2, 128]:
    A[d2t, d1p, p, ko, d2p] = SC_SCALE * M[(2p+ko)*128+d1p, d2t*128+d2p]."""
    Wqp = np.zeros((D, DP), dtype=np.float32)
    Wqp[:, :D] = Wq
    Wqp[:, D] = bq
    Wkp = np.zeros((D, DP), dtype=np.float32)
    Wkp[:, :D] = Wk
    Wkp[:, D] = bk
    M = (Wqp.T @ Wkp) * np.float32(SC_SCALE / np.sqrt(np.float64(D)))
    A = M.reshape(PAIRS, 2, 128, ET, 128).transpose(3, 2, 0, 1, 4)
    return np.ascontiguousarray(A, dtype=F8NP)


def _pack_zt(W1, b1, Wv, bv):
    """Z = W1' @ Wv' [H, DP]: the v projection and the first MLP layer
    fused. Wv' carries bv in column d=3800 plus a unit at [3800, 3800] so
    x's bias feature flows through; b1 is added into Z's column 3800.
    Packed for the DR moving operand: zt[d1p, pair, ko, o] =
    ZSC * Z[o, (2*pair+ko)*128 + d1p] -> [128, PAIRS, 2, H]."""
    Wvp = np.zeros((DP, DP), dtype=np.float32)
    Wvp[:D, :D] = Wv
    Wvp[:D, D] = bv
    Wvp[D, D] = 1.0
    W1p = np.zeros((H, DP), dtype=np.float32)
    W1p[:, :D] = W1
    W1p[:, D] = b1
    Z = (W1p @ Wvp) * np.float32(ZSC)  # [H, DP]
    A = Z.T.reshape(PAIRS, 2, 128, H).transpose(2, 0, 1, 3)
    return np.ascontiguousarray(A, dtype=F8NP)


def _pack_x8(xc):
    """xc [BLOC, S, D] -> [128, PAIRS, 2, T] e4m3, bias row d=3800 = 1."""
    xt = np.zeros((DP, T), dtype=np.float32)
    xt[:D, :] = xc.reshape(T, D).T
    xt[D, :] = 1.0
    A = xt.reshape(PAIRS, 2, 128, T).transpose(2, 0, 1, 3)
    return np.ascontiguousarray(A, dtype=F8NP)


def _pack_w2(W2, b2):
    A = np.zeros((128, 5, H), dtype=np.float32)
    A[:, :4, :] = W2.T.reshape(4, 128, H).transpose(1, 0, 2)
    A[0, 4, :] = b2
    return np.ascontiguousarray(A)


def _pack_w3(W3, b3):
    A = np.zeros((128, 5, C), dtype=np.float32)
    A[:, :4, :] = W3.T.reshape(4, 128, C).transpose(1, 0, 2)
    A[0, 4, :] = b3
    return np.ascontiguousarray(A)


_NC_CACHE = {}


def _get_nc():
    if "nc" not in _NC_CACHE:
        _NC_CACHE["nc"] = build_kernel()
    return _NC_CACHE["nc"]


def kernel(x, Wk, bk, Wq, bq, Wv, bv, W1, b1, W2, b2, W3, b3, _trace=False):
    x = np.asarray(x, dtype=np.float32)

    m8_p = _pack_m8(
        np.asarray(Wq, np.float32), np.asarray(bq, np.float32),
        np.asarray(Wk, np.float32), np.asarray(bk, np.float32),
    )
    zt_p = _pack_zt(
        np.asarray(W1, np.float32), np.asarray(b1, np.float32),
        np.asarray(Wv, np.float32), np.asarray(bv, np.float32),
    )
    w2_p = _pack_w2(np.asarray(W2, np.float32), np.asarray(b2, np.float32))
    w3_p = _pack_w3(np.asarray(W3, np.float32), np.asarray(b3, np.float32))
    e0b = np.zeros((128, BLOC), dtype=np.float32)
    e0b[0, :] = 1.0

    in_maps = []
    for c in range(N_CORES):
        xc = x[c * BLOC : (c + 1) * BLOC]
        in_maps.append(
            {
                "x8": _pack_x8(xc),
                "m8": m8_p,
                "zt": zt_p,
                "w2": w2_p,
                "w3": w3_p,
                "e0b": e0b,
            }
        )

    nc = _get_nc()
    _install_verbose_cc_hook()
    res = run_bass_kernel_spmd(nc, in_maps, list(range(N_CORES)), trace=_trace)
    out = np.zeros((B, C), dtype=np.float32)
    for c in range(N_CORES):
        out[c * BLOC : (c + 1) * BLOC] = res.results[c]["outT"].T
    if _trace:
        return out, res
    return out

